# revision 3
# baseline (speedup 1.0000x reference)
"""Causal self-attention (B=4, T=2048, C=1024, H=16) on 8 TRN2 NeuronCores.

Sharding: core = 2*b + hg handles batch b and heads hg*8..hg*8+8 (hybrid
batch x tensor parallel). Each core computes QKV for its heads, causal
attention, and a partial output projection over its 512 y-columns.
Host sums the two partials per batch, divides by 32 (fp8 weight
pre-scale), transposes, and adds b_proj + w_proj @ b_v.

v3: the Q/K projections for token spans 1..3 run in fp8e4m3 with the
DoubleRow perf mode (0.5 cycles/row; both operands carry 2 contraction
rows per partition as a [128, 2, N] AP) — those QKV groups drop 4x in
PE time. fp8 is ONLY safe on the score path AND only for long rows:
softmax renormalization damps score errors by ~sqrt(row length), so
~5% per-element fp8 noise in Q/K becomes <0.4% in y for tokens >=512,
but stays ~2.5% for early tokens (measured 2.9e-2 rel err with span0
in fp8 too, vs the 2e-2 tolerance; 4.9e-2 with everything fp8). So
span 0 Q/K runs in bf16, and the V path and output projection stay
bf16 (their errors reach the output undamped). x is staged twice (fp8
for spans 1-3 Q/K, bf16 for V and span0). fp8 Q/K weights are
pre-scaled x32 into the fp8 normal range; the fp8 drains rescale with
a fused tensor_scalar (mult + per-partition bias add).

v2 design notes (vs the v1 kernel):
  * bf16/fp8 device inputs (DMA is a single serialized ~360GB/s stream,
    so fewer bytes shorten the startup critical path; bf16/fp8 matmuls
    also run 1 cycle/row at ANY free size, while f32r pays 4x below
    256 free — the diagonal S blocks).
  * single-stream attention per head pair with a 2-deep S-tile pipeline
    (S(kt+2) issued while exp(kt+1) runs and PV(kt) consumes).
  * causal mask applied on PE as an extra accumulated matmul
    (idn^T @ msk into the diagonal 128-col strip of the S psum tile).
  * dedicated PSUM pools: S 2x2 banks, y 2 banks, fillers 2x1 banks.
  * DMA strictly ordered by first use; wqk grouped by feature block so
    the first QKV groups need only 2 of the 8 wqk tiles.
  * y_ps copied PSUM->SBUF right after the last PV so the y psum frees
    in ~1.2us instead of after the whole norm chain.
  * exp table preloaded at t~0; V ones-columns prefilled once.
  * fillers: seg0 carries span0-leftovers + span1 QKV, seg1/2 carry
    span2/3 QKV, seg3 carries ALL of proj0..2 (it is ACT-heavy);
    proj3 is the tail.

Device-side layouts (t=token, c=embed, f=qkv feature, d=head dim):
  xTf8 [1024, 2048]     x[b].T                             (fp8e4)
  xTbf [1024, 2048]     x[b].T                             (bf16)
  wqkg [8, 128, 8, 128] wqkg[mf][i, cc, j] = wqk[cc*128+i, mf*128+j],
                        wqk cols: 512 Q (x32 net), 512 K (x32) (fp8e4)
  wqkbg [8, 128, 8, 128] same layout, unscaled (Q /8)       (bf16)
  wv   [1024, 512]                                          (bf16)
  wp   [512, 1024]      w_proj[:, my_cols].T                (bf16)
  bqk  [128, 8]         per f-tile bias columns (Q part /8)  (f32)
  msk  [128, 128]       0 where i<=j else -1e30              (bf16)
  idn  [128, 128]       identity                             (bf16)
Output: outT [1024, 2048] partial out^T (no bias)            (f32)

Attention per head pair p (heads 2p, 2p+1 on partitions 0:64 / 64:128
of QT[p], KT[p]); per q-block qb (512 q), per kt (128 k):
  S^T[tk, tq] = K_blk @ Q^T      (2 row-packed K=64 bf16 matmuls)
  (+ diag strip: idn^T @ msk accumulated in psum — causal mask)
  P = exp(S^T) -> bf16           (no max subtraction; scores ~ N(0,1))
  [y_un^T; den] = V_aug^T @ P    (V_aug bf16 with a ones column, M=65)
  y^T = y_un^T * partition_broadcast(1/den)     (at pair end)
Projection: out^T[o, t] = wp^T @ y^T accumulated over 4 c-chunks.
"""
from contextlib import ExitStack

import numpy as np

N_HEAD = 16
C = 1024
B = 4
T = 2048
D = 64
NCC = C // 128  # c chunks
NTT = T // 128  # t tiles
NTS = T // 512  # t spans / q blocks

_CACHE = {}


def _build_nc(reps=1, mode="full", pbufs=3):
    import concourse.mybir as mybir
    import concourse.tile as tile
    from concourse import bacc

    f32 = mybir.dt.float32
    bf16 = mybir.dt.bfloat16
    fp8 = mybir.dt.float8e4
    DR = mybir.MatmulPerfMode.DoubleRow
    MUL = mybir.AluOpType.mult
    ADD = mybir.AluOpType.add

    nc = bacc.Bacc()
    xTf8 = nc.declare_dram_parameter("xTf8", [C, T], fp8, isOutput=False)
    xTbf = nc.declare_dram_parameter("xTbf", [C, T], bf16, isOutput=False)
    wqkg = nc.declare_dram_parameter("wqkg", [8, 128, 8, 128], fp8, isOutput=False)
    wqkbg = nc.declare_dram_parameter("wqkbg", [8, 128, 8, 128], bf16, isOutput=False)
    wv = nc.declare_dram_parameter("wv", [C, 512], bf16, isOutput=False)
    wp = nc.declare_dram_parameter("wp", [512, C], bf16, isOutput=False)
    bqk = nc.declare_dram_parameter("bqk", [128, 8], f32, isOutput=False)
    msk = nc.declare_dram_parameter("msk", [128, 128], bf16, isOutput=False)
    idn = nc.declare_dram_parameter("idn", [128, 128], bf16, isOutput=False)
    outT = nc.declare_dram_parameter("outT", [C, T], bf16, isOutput=True)

    Exp = mybir.ActivationFunctionType.Exp

    with tile.TileContext(nc) as tc, ExitStack() as ctx:
        persist = ctx.enter_context(tc.tile_pool(name="persist", bufs=1))
        wpool = ctx.enter_context(tc.tile_pool(name="wpool", bufs=1))
        xpool = ctx.enter_context(tc.tile_pool(name="xpool", bufs=2))
        ppool = ctx.enter_context(tc.tile_pool(name="ppool", bufs=pbufs))
        npool = ctx.enter_context(tc.tile_pool(name="npool", bufs=1))
        opool = ctx.enter_context(tc.tile_pool(name="opool", bufs=2))
        pss = ctx.enter_context(tc.tile_pool(name="pss", bufs=3, space="PSUM"))
        psy = ctx.enter_context(tc.tile_pool(name="psy", bufs=1, space="PSUM"))

        bqk_sb = persist.tile([128, 8], f32)
        msk_sb = persist.tile([128, 128], bf16)
        idn_sb = persist.tile([128, 128], bf16)
        warm = persist.tile([128, 1], f32)
        warm_o = persist.tile([128, 1], bf16)

        QT = [persist.tile([128, T], bf16, tag=f"qt{p}", name=f"qt{p}") for p in range(4)]
        KT = [persist.tile([128, T], bf16, tag=f"kt{p}", name=f"kt{p}") for p in range(4)]
        V = [persist.tile([128, 8, 65], bf16, tag=f"v{tt}", name=f"v{tt}") for tt in range(NTT)]
        Y = [persist.tile([128, T], bf16, tag=f"y{p}", name=f"y{p}") for p in range(4)]

        # preload the Exp activation table + V ones-columns while DMAs run
        nc.vector.memset(warm, 0.0)
        nc.scalar.activation(out=warm_o, in_=warm, func=Exp)
        for tt in range(NTT):
            nc.vector.memset(V[tt][:, :, 64:65], 1.0)

        # ---- DMA priming, strictly in first-use order ----
        wqk_sb = [None] * 8
        wqkb_sb = [None] * 8

        def load_wqk(mf):
            t_ = wpool.tile([128, 8, 128], fp8, tag=f"wqk{mf}", name=f"wqk{mf}")
            nc.sync.dma_start(out=t_, in_=wqkg[mf, :, :, :])
            wqk_sb[mf] = t_

        def load_wqkb(mf):
            t_ = wpool.tile([128, 8, 128], bf16, tag=f"wqkb{mf}", name=f"wqkb{mf}")
            nc.sync.dma_start(out=t_, in_=wqkbg[mf, :, :, :])
            wqkb_sb[mf] = t_

        load_wqkb(0)

        def load_xspan(ts):
            x8 = []
            for u in range(4 if ts > 0 else 0):
                t_ = xpool.tile([128, 2, 512], fp8, tag=f"x8{u}", name=f"x8{u}")
                for i in range(2):
                    nc.sync.dma_start(
                        out=t_[:, i, :],
                        in_=xTf8[(2 * u + i) * 128:(2 * u + i + 1) * 128,
                                 ts * 512:(ts + 1) * 512],
                    )
                x8.append(t_)
            xb = []
            for cc in range(NCC):
                t_ = xpool.tile([128, 512], bf16, tag=f"xb{cc}", name=f"xb{cc}")
                nc.sync.dma_start(
                    out=t_,
                    in_=xTbf[cc * 128:(cc + 1) * 128, ts * 512:(ts + 1) * 512],
                )
                xb.append(t_)
            return x8, xb

        xs = load_xspan(0)
        load_wqkb(4)
        nc.sync.dma_start(out=bqk_sb, in_=bqk[:, :])
        nc.sync.dma_start(out=msk_sb, in_=msk[:, :])
        nc.sync.dma_start(out=idn_sb, in_=idn[:, :])
        wv_sb = []
        for cc in range(NCC):
            t_ = wpool.tile([128, 512], bf16, tag=f"wv{cc}", name=f"wv{cc}")
            nc.sync.dma_start(out=t_, in_=wv[cc * 128:(cc + 1) * 128, :])
            wv_sb.append(t_)
        for mf in (1, 5, 2, 6, 3, 7):
            load_wqkb(mf)
        for mf in range(8):
            load_wqk(mf)

        wp_sb = []

        def load_wp():
            for cc in range(4):
                t_ = wpool.tile([128, 1024], bf16, tag=f"wp{cc}", name=f"wp{cc}")
                nc.sync.dma_start(out=t_, in_=wp[cc * 128:(cc + 1) * 128, :])
                wp_sb.append(t_)

        def qk_group(ts, xs, mf):
            """One QT/KT projection group. Span 0 runs bf16 (8 chained
            matmuls — short attention rows can't afford fp8 noise);
            spans 1-3 run fp8 DoubleRow (4 chained)."""
            x8, xb = xs
            pq = pss.tile([128, 512], f32, tag="s", name="pq",
                          padded_shape=[128, 1024])
            dst = QT[mf] if mf < 4 else KT[mf - 4]
            if ts == 0:
                for cc in range(NCC):
                    nc.tensor.matmul(
                        pq,
                        wqkb_sb[mf][:, cc, :],
                        xb[cc],
                        start=(cc == 0),
                        stop=(cc == NCC - 1),
                    )
                nc.vector.tensor_scalar_add(
                    dst[:, 0:512], pq, bqk_sb[:, mf:mf + 1]
                )
                return
            for u in range(4):
                nc.tensor.matmul(
                    pq,
                    wqk_sb[mf][:, 2 * u:2 * u + 2, :],
                    x8[u],
                    start=(u == 0),
                    stop=(u == 3),
                    perf_mode=DR,
                )
            # undo the x32 fp8 weight pre-scale (Q also carries /8): the
            # drain computes dst = pq * s + bias in one DVE op
            s = 1.0 / 256.0 if mf < 4 else 1.0 / 32.0
            nc.vector.tensor_scalar(
                dst[:, ts * 512:(ts + 1) * 512], pq,
                s, bqk_sb[:, mf:mf + 1], MUL, ADD,
            )

        def v_group(ts, xs, tt4):
            """One V projection matmul group (8 chained, bf16)."""
            xb = xs[1]
            tt = ts * 4 + tt4
            pv = pss.tile([128, 512], f32, tag="s", name="pv",
                          padded_shape=[128, 1024])
            for cc in range(NCC):
                nc.tensor.matmul(
                    pv,
                    xb[cc][:, tt4 * 128:(tt4 + 1) * 128],
                    wv_sb[cc],
                    start=(cc == 0),
                    stop=(cc == NCC - 1),
                )
            nc.vector.tensor_copy(
                out=V[tt][:, :, 0:64],
                in_=pv.rearrange("p (h d) -> p h d", h=8),
            )

        def qkv_groups(ts, xs, skip_first6=False):
            """Matmul groups of the QKV projection for t-span ts, ordered so
            pair p's QT/KT and the span's V come up in consumption order."""
            order = [("qk", 0), ("qk", 4), ("v", 0), ("v", 1), ("v", 2),
                     ("v", 3), ("qk", 1), ("qk", 5), ("qk", 2), ("qk", 6),
                     ("qk", 3), ("qk", 7)]
            if skip_first6:
                order = order[6:]
            for kind, i in order:
                if kind == "qk":
                    yield lambda i=i: qk_group(ts, xs, i)
                else:
                    yield lambda i=i: v_group(ts, xs, i)

        def proj_group(ts, mo):
            """One output-projection matmul group (4 chained, bf16)."""
            po = pss.tile([128, 512], f32, tag="s", name="po",
                          padded_shape=[128, 1024])
            for cc in range(4):
                nc.tensor.matmul(
                    po,
                    wp_sb[cc][:, mo * 128:(mo + 1) * 128],
                    Y[cc][:, ts * 512:(ts + 1) * 512],
                    start=(cc == 0),
                    stop=(cc == 3),
                )
            ot = opool.tile([128, 512], bf16, tag="ot", name="ot")
            nc.vector.tensor_copy(ot, po)
            nc.sync.dma_start(
                out=outT[mo * 128:(mo + 1) * 128, ts * 512:(ts + 1) * 512],
                in_=ot,
            )

        def proj_groups(ts):
            for mo in range(8):
                yield lambda mo=mo: proj_group(ts, mo)

        class Fillers:
            """One list per segment; pair prologues draw 2 at each boundary,
            the rest are spread proportionally over kt iterations."""

            def __init__(self, groups, n_iter, reserve=0):
                self.groups = list(groups)
                self.n_iter = max(1, n_iter)
                self.emitted = 0
                self.bdry = 0
                self.it = 0
                self.spread = max(0, len(self.groups) - 8 - reserve)

            def boundary(self, k=2):
                for _ in range(k):
                    if self.emitted < len(self.groups):
                        self.groups[self.emitted]()
                        self.emitted += 1
                        self.bdry += 1

            def step(self):
                self.it += 1
                want = self.bdry + (self.it * self.spread) // self.n_iter
                while self.emitted < min(want, len(self.groups)):
                    self.groups[self.emitted]()
                    self.emitted += 1

            def drain(self):
                while self.emitted < len(self.groups):
                    self.groups[self.emitted]()
                    self.emitted += 1

        def pair_attention(qb, p, fil):
            """Attention for q-block qb, head pair p, single-stream with a
            2-deep S pipeline: per kt the PE does PV(kt) then S(kt+2),
            ACT does exp(kt+1)."""
            q0 = 512 * qb
            n_kt = 4 * qb + 4
            y_ps = psy.tile([65, 1024], f32, tag="y", name="y_ps")

            def s_stage(kt):
                s_t = pss.tile([128, 2, 512], f32, tag="s", name="s_t")
                c_lo = max(0, 128 * kt - q0)
                d0 = 128 * kt - q0
                diag = d0 >= 0
                nc.tensor.matmul(
                    s_t[:, 0, c_lo:512],
                    KT[p][0:64, 128 * kt:128 * kt + 128],
                    QT[p][0:64, q0 + c_lo:q0 + 512],
                    start=True, stop=not diag,
                )
                nc.tensor.matmul(
                    s_t[:, 1, c_lo:512],
                    KT[p][64:128, 128 * kt:128 * kt + 128],
                    QT[p][64:128, q0 + c_lo:q0 + 512],
                    start=True, stop=not diag,
                )
                if diag:
                    nc.tensor.matmul(
                        s_t[:, 0, d0:d0 + 128], idn_sb, msk_sb,
                        start=False, stop=True,
                    )
                    nc.tensor.matmul(
                        s_t[:, 1, d0:d0 + 128], idn_sb, msk_sb,
                        start=False, stop=True,
                    )
                return s_t

            def exp_stage(kt, s_t):
                c_lo = max(0, 128 * kt - q0)
                p_t = ppool.tile([128, 2, 512], bf16, tag="p", name="p_t")
                if mode != "noexp":
                    nc.scalar.activation(
                        out=p_t[:, :, c_lo:], in_=s_t[:, :, c_lo:], func=Exp
                    )
                else:
                    nc.vector.memset(p_t[:, :, c_lo:], 0.5)
                return p_t

            def pv_stage(kt, p_t):
                c_lo = max(0, 128 * kt - q0)
                nc.tensor.matmul(
                    y_ps[:, c_lo:512], V[kt][:, 2 * p, :], p_t[:, 0, c_lo:],
                    start=(kt == 0), stop=(kt == n_kt - 1),
                )
                nc.tensor.matmul(
                    y_ps[:, 512 + c_lo:1024], V[kt][:, 2 * p + 1, :],
                    p_t[:, 1, c_lo:],
                    start=(kt == 0), stop=(kt == n_kt - 1),
                )

            # prologue: boundary fillers cover prev pair's y-psum reuse and
            # this pair's S0 -> exp0 -> PV0 latency. exp runs 2 iterations
            # ahead of its PV consumer (3-deep S pipeline) so ACT jitter and
            # cross-engine semaphore latency never gate the PE.
            fil.boundary(2)
            s_tiles = [None] * n_kt
            p_tiles = [None] * n_kt
            s_tiles[0] = s_stage(0)
            p_tiles[0] = exp_stage(0, s_tiles[0])
            if n_kt > 1:
                s_tiles[1] = s_stage(1)
                p_tiles[1] = exp_stage(1, s_tiles[1])
            if n_kt > 2:
                s_tiles[2] = s_stage(2)
            for kt in range(n_kt):
                pv_stage(kt, p_tiles[kt])
                p_tiles[kt] = None
                if kt + 3 < n_kt:
                    s_tiles[kt + 3] = s_stage(kt + 3)
                    s_tiles[kt] = None
                if kt + 2 < n_kt:
                    p_tiles[kt + 2] = exp_stage(kt + 2, s_tiles[kt + 2])
                fil.step()

            # y_ps -> SBUF copy frees the y psum quickly; then normalize:
            # y = y_un * bcast(1/den). The very last pair skips the copy
            # (nothing reuses the y psum, and the tail waits on this chain).
            if qb == NTS - 1 and p == 3:
                ycp = y_ps
            else:
                ycp = npool.tile([65, 1024], f32, tag="ycp", name="ycp")
                nc.vector.tensor_copy(ycp, y_ps)
            rb = npool.tile([64, 1024], f32, tag="rb", name="rb")
            nc.vector.reciprocal(rb[0:1, :], ycp[64:65, :])
            nc.gpsimd.partition_broadcast(rb, rb[0:1, :])
            nc.vector.tensor_mul(
                Y[p][0:64, q0:q0 + 512], ycp[0:64, 0:512], rb[:, 0:512]
            )
            nc.vector.tensor_mul(
                Y[p][64:128, q0:q0 + 512], ycp[0:64, 512:1024], rb[:, 512:1024]
            )

        # ---- prologue + pipelined segments ----
        def body(xs0):
            xs = xs0 if xs0 is not None else load_xspan(0)
            # span-0 groups needed by seg0 pair0: qk0, qk4, v0..v3
            pro = list(qkv_groups(0, xs))
            for g in pro[:6]:
                g()
            for qb in range(NTS):
                groups = []
                if qb == 0:
                    groups.extend(pro[6:])
                if qb + 1 < NTS:
                    xs = load_xspan(qb + 1)
                    if qb == 0:
                        load_wp()
                    groups.extend(qkv_groups(qb + 1, xs))
                else:
                    for ts in range(3):
                        groups.extend(proj_groups(ts))
                fil = Fillers(groups, 4 * (4 * qb + 4),
                              reserve=4 if qb == NTS - 1 else 0)
                for pr in range(4):
                    pair_attention(qb, pr, fil)
                fil.drain()
            for g in proj_groups(3):
                g()

        if reps == 1:
            body(xs)
        else:
            ET = mybir.EngineType
            with tc.For_i(0, reps, 1,
                          hint_engines=(ET.PE, ET.DVE, ET.Activation, ET.SP, ET.Pool)):
                body(None)

    nc.finalize()
    return nc


def _prep_core_inputs(x, w_attn, b_attn, w_proj, core):
    import ml_dtypes

    fp8 = ml_dtypes.float8_e4m3
    bf16 = ml_dtypes.bfloat16
    b, hg = core // 2, core % 2
    s = hg * 512
    xT = np.ascontiguousarray(x[b].T)
    # x32 pre-scale moves the small Q/K weights into fp8e4m3's normal range
    wq = w_attn[s:s + 512] * 32.0
    wk = w_attn[1024 + s:1024 + s + 512] * 32.0
    wqk = np.concatenate([wq, wk], axis=0).T  # [c, f]
    wqkg = np.ascontiguousarray(
        wqk.reshape(8, 128, 8, 128).transpose(2, 1, 0, 3))
    wqkb = np.concatenate(
        [w_attn[s:s + 512] * 0.125, w_attn[1024 + s:1024 + s + 512]],
        axis=0).T
    wqkbg = np.ascontiguousarray(
        wqkb.reshape(8, 128, 8, 128).transpose(2, 1, 0, 3))
    wv = np.ascontiguousarray(w_attn[2048 + s:2048 + s + 512].T)
    wp = np.ascontiguousarray(w_proj[:, s:s + 512].T)
    bq = b_attn[s:s + 512] * 0.125
    bk = b_attn[1024 + s:1024 + s + 512]
    bqk = np.ascontiguousarray(np.concatenate([bq, bk]).reshape(8, 128).T)
    r = np.arange(128, dtype=np.int32)
    msk = np.where(r[:, None] <= r[None, :], 0.0, -1e30)
    idn = np.eye(128)
    return {
        "xTf8": xT.astype(fp8),
        "xTbf": xT.astype(bf16),
        "wqkg": wqkg.astype(fp8),
        "wqkbg": wqkbg.astype(bf16),
        "wv": wv.astype(bf16),
        "wp": wp.astype(bf16),
        "bqk": bqk.astype(np.float32),
        "msk": msk.astype(bf16),
        "idn": idn.astype(bf16),
    }


def kernel(x, w_attn, b_attn, w_proj, b_proj):
    from concourse.bass_utils import run_bass_kernel_spmd

    x = np.asarray(x, dtype=np.float32)
    w_attn = np.asarray(w_attn, dtype=np.float32)
    b_attn = np.asarray(b_attn, dtype=np.float32)
    w_proj = np.asarray(w_proj, dtype=np.float32)
    b_proj = np.asarray(b_proj, dtype=np.float32)

    if "nc" not in _CACHE:
        _CACHE["nc"] = _build_nc()
    nc = _CACHE["nc"]

    in_maps = [
        _prep_core_inputs(x, w_attn, b_attn, w_proj, core) for core in range(8)
    ]
    res = run_bass_kernel_spmd(nc, in_maps, list(range(8)))

    # total bias: b_proj plus the (token-constant) V-bias contribution
    bias = b_proj + w_proj @ b_attn[2048:]
    out = np.empty((B, T, C), dtype=np.float32)
    for b in range(B):
        acc = (res.results[2 * b]["outT"].astype(np.float32)
               + res.results[2 * b + 1]["outT"].astype(np.float32))
        out[b] = acc.T + bias
    return out


# revision 6
# speedup vs baseline: 1.1908x; 1.1908x over previous
"""Causal self-attention (B=4, T=2048, C=1024, H=16) on 8 TRN2 NeuronCores.

Sharding: core = 2*b + hg handles batch b and heads hg*8..hg*8+8 (hybrid
batch x tensor parallel). Each core computes QKV for its heads, causal
attention, and a partial output projection over its 512 y-columns.
Host sums the two partials per batch, divides by 32 (fp8 weight
pre-scale), transposes, and adds b_proj + w_proj @ b_v.

v3: the Q/K projections for token spans 1..3 run in fp8e4m3 with the
DoubleRow perf mode (0.5 cycles/row; both operands carry 2 contraction
rows per partition as a [128, 2, N] AP) — those QKV groups drop 4x in
PE time. fp8 is ONLY safe on the score path AND only for long rows:
softmax renormalization damps score errors by ~sqrt(row length), so
~5% per-element fp8 noise in Q/K becomes <0.4% in y for tokens >=512,
but stays ~2.5% for early tokens (measured 2.9e-2 rel err with span0
in fp8 too, vs the 2e-2 tolerance; 4.9e-2 with everything fp8). So
span 0 Q/K runs in bf16, and the V path and output projection stay
bf16 (their errors reach the output undamped). x is staged twice (fp8
for spans 1-3 Q/K, bf16 for V and span0). fp8 Q/K weights are
pre-scaled x32 into the fp8 normal range; the fp8 drains rescale with
a fused tensor_scalar (mult + per-partition bias add).

v2 design notes (vs the v1 kernel):
  * bf16/fp8 device inputs (DMA is a single serialized ~360GB/s stream,
    so fewer bytes shorten the startup critical path; bf16/fp8 matmuls
    also run 1 cycle/row at ANY free size, while f32r pays 4x below
    256 free — the diagonal S blocks).
  * single-stream attention per head pair with a 2-deep S-tile pipeline
    (S(kt+2) issued while exp(kt+1) runs and PV(kt) consumes).
  * causal mask applied on PE as an extra accumulated matmul
    (idn^T @ msk into the diagonal 128-col strip of the S psum tile).
  * dedicated PSUM pools: S 2x2 banks, y 2 banks, fillers 2x1 banks.
  * DMA strictly ordered by first use; wqk grouped by feature block so
    the first QKV groups need only 2 of the 8 wqk tiles.
  * y_ps copied PSUM->SBUF right after the last PV so the y psum frees
    in ~1.2us instead of after the whole norm chain.
  * exp table preloaded at t~0; V ones-columns prefilled once.
  * fillers: seg0 carries span0-leftovers + span1 QKV, seg1/2 carry
    span2/3 QKV, seg3 carries ALL of proj0..2 (it is ACT-heavy);
    proj3 is the tail.

Device-side layouts (t=token, c=embed, f=qkv feature, d=head dim):
  xTf8 [1024, 2048]     x[b].T                             (fp8e4)
  xTbf [1024, 2048]     x[b].T                             (bf16)
  wqkg [8, 128, 8, 128] wqkg[mf][i, cc, j] = wqk[cc*128+i, mf*128+j],
                        wqk cols: 512 Q (x32 net), 512 K (x32) (fp8e4)
  wqkbg [8, 128, 8, 128] same layout, unscaled (Q /8)       (bf16)
  wv   [1024, 512]                                          (bf16)
  wp   [512, 1024]      w_proj[:, my_cols].T                (bf16)
  bqk  [128, 8]         per f-tile bias columns (Q part /8)  (f32)
  msk  [128, 128]       0 where i<=j else -1e30              (bf16)
  idn  [128, 128]       identity                             (bf16)
Output: outT [1024, 2048] partial out^T (no bias)            (f32)

Attention per head pair p (heads 2p, 2p+1 on partitions 0:64 / 64:128
of QT[p], KT[p]); per q-block qb (512 q), per kt (128 k):
  S^T[tk, tq] = K_blk @ Q^T      (2 row-packed K=64 bf16 matmuls)
  (+ diag strip: idn^T @ msk accumulated in psum — causal mask)
  P = exp(S^T) -> bf16           (no max subtraction; scores ~ N(0,1))
  [y_un^T; den] = V_aug^T @ P    (V_aug bf16 with a ones column, M=65)
  y^T = y_un^T * partition_broadcast(1/den)     (at pair end)
Projection: out^T[o, t] = wp^T @ y^T accumulated over 4 c-chunks.
"""
from contextlib import ExitStack

import numpy as np

N_HEAD = 16
C = 1024
B = 4
T = 2048
D = 64
NCC = C // 128  # c chunks
NTT = T // 128  # t tiles
NTS = T // 512  # t spans / q blocks

_CACHE = {}


def _build_nc(reps=1, mode="full", pbufs=3):
    import concourse.mybir as mybir
    import concourse.tile as tile
    from concourse import bacc

    f32 = mybir.dt.float32
    bf16 = mybir.dt.bfloat16
    fp8 = mybir.dt.float8e4
    DR = mybir.MatmulPerfMode.DoubleRow
    MUL = mybir.AluOpType.mult
    ADD = mybir.AluOpType.add

    nc = bacc.Bacc()
    xTf8 = nc.declare_dram_parameter("xTf8", [C, T], fp8, isOutput=False)
    xTbf = nc.declare_dram_parameter("xTbf", [C, T], bf16, isOutput=False)
    wqkg = nc.declare_dram_parameter("wqkg", [8, 128, 8, 128], fp8, isOutput=False)
    wqkbg = nc.declare_dram_parameter("wqkbg", [8, 128, 8, 128], bf16, isOutput=False)
    wv = nc.declare_dram_parameter("wv", [C, 512], bf16, isOutput=False)
    wp = nc.declare_dram_parameter("wp", [512, C], bf16, isOutput=False)
    bqk = nc.declare_dram_parameter("bqk", [128, 8], f32, isOutput=False)
    msk = nc.declare_dram_parameter("msk", [128, 128], bf16, isOutput=False)
    idn = nc.declare_dram_parameter("idn", [128, 128], bf16, isOutput=False)
    outT = nc.declare_dram_parameter("outT", [C, T], bf16, isOutput=True)

    Exp = mybir.ActivationFunctionType.Exp

    with tile.TileContext(nc) as tc, ExitStack() as ctx:
        persist = ctx.enter_context(tc.tile_pool(name="persist", bufs=1))
        wpool = ctx.enter_context(tc.tile_pool(name="wpool", bufs=1))
        xpool = ctx.enter_context(tc.tile_pool(name="xpool", bufs=2))
        ppool = ctx.enter_context(tc.tile_pool(name="ppool", bufs=pbufs))
        npool = ctx.enter_context(tc.tile_pool(name="npool", bufs=1))
        opool = ctx.enter_context(tc.tile_pool(name="opool", bufs=2))
        pss = ctx.enter_context(tc.tile_pool(name="pss", bufs=2, space="PSUM"))
        psy = ctx.enter_context(tc.tile_pool(name="psy", bufs=1, space="PSUM"))
        psf = ctx.enter_context(tc.tile_pool(name="psf", bufs=2, space="PSUM"))

        bqk_sb = persist.tile([128, 8], f32)
        msk_sb = persist.tile([128, 128], bf16)
        idn_sb = persist.tile([128, 128], bf16)
        warm = persist.tile([128, 1], f32)
        warm_o = persist.tile([128, 1], bf16)

        QT = [persist.tile([128, T], bf16, tag=f"qt{p}", name=f"qt{p}") for p in range(4)]
        KT = [persist.tile([128, T], bf16, tag=f"kt{p}", name=f"kt{p}") for p in range(4)]
        V = [persist.tile([128, 8, 65], bf16, tag=f"v{tt}", name=f"v{tt}") for tt in range(NTT)]
        Y = [persist.tile([128, T], bf16, tag=f"y{p}", name=f"y{p}") for p in range(4)]

        # preload the Exp activation table + V ones-columns while DMAs run
        nc.vector.memset(warm, 0.0)
        nc.scalar.activation(out=warm_o, in_=warm, func=Exp)
        for tt in range(NTT):
            nc.vector.memset(V[tt][:, :, 64:65], 1.0)

        # ---- DMA priming, strictly in first-use order ----
        wqk_sb = [None] * 8
        wqkb_sb = [None] * 8

        def load_wqk(mf):
            t_ = wpool.tile([128, 8, 128], fp8, tag=f"wqk{mf}", name=f"wqk{mf}")
            nc.sync.dma_start(out=t_, in_=wqkg[mf, :, :, :])
            wqk_sb[mf] = t_

        def load_wqkb(mf):
            t_ = wpool.tile([128, 8, 128], bf16, tag=f"wqkb{mf}", name=f"wqkb{mf}")
            nc.sync.dma_start(out=t_, in_=wqkbg[mf, :, :, :])
            wqkb_sb[mf] = t_

        load_wqkb(0)

        def load_xspan(ts):
            x8 = []
            for u in range(4 if ts > 0 else 0):
                t_ = xpool.tile([128, 2, 512], fp8, tag=f"x8{u}", name=f"x8{u}")
                for i in range(2):
                    nc.sync.dma_start(
                        out=t_[:, i, :],
                        in_=xTf8[(2 * u + i) * 128:(2 * u + i + 1) * 128,
                                 ts * 512:(ts + 1) * 512],
                    )
                x8.append(t_)
            xb = []
            for cc in range(NCC):
                t_ = xpool.tile([128, 512], bf16, tag=f"xb{cc}", name=f"xb{cc}")
                nc.sync.dma_start(
                    out=t_,
                    in_=xTbf[cc * 128:(cc + 1) * 128, ts * 512:(ts + 1) * 512],
                )
                xb.append(t_)
            return x8, xb

        xs = load_xspan(0)
        load_wqkb(4)
        nc.sync.dma_start(out=bqk_sb, in_=bqk[:, :])
        nc.sync.dma_start(out=msk_sb, in_=msk[:, :])
        nc.sync.dma_start(out=idn_sb, in_=idn[:, :])
        wv_sb = []
        for cc in range(NCC):
            t_ = wpool.tile([128, 512], bf16, tag=f"wv{cc}", name=f"wv{cc}")
            nc.sync.dma_start(out=t_, in_=wv[cc * 128:(cc + 1) * 128, :])
            wv_sb.append(t_)
        for mf in (1, 5, 2, 6, 3, 7):
            load_wqkb(mf)
        for mf in range(8):
            load_wqk(mf)

        wp_sb = []

        def load_wp():
            for cc in range(4):
                t_ = wpool.tile([128, 1024], bf16, tag=f"wp{cc}", name=f"wp{cc}")
                nc.sync.dma_start(out=t_, in_=wp[cc * 128:(cc + 1) * 128, :])
                wp_sb.append(t_)

        def qk_group(ts, xs, mf):
            """One QT/KT projection group. Span 0 runs bf16 (8 chained
            matmuls — short attention rows can't afford fp8 noise);
            spans 1-3 run fp8 DoubleRow (4 chained)."""
            x8, xb = xs
            pq = psf.tile([128, 512], f32, tag="f", name="pq")
            dst = QT[mf] if mf < 4 else KT[mf - 4]
            if ts == 0:
                for cc in range(NCC):
                    nc.tensor.matmul(
                        pq,
                        wqkb_sb[mf][:, cc, :],
                        xb[cc],
                        start=(cc == 0),
                        stop=(cc == NCC - 1),
                    )
                nc.vector.tensor_scalar_add(
                    dst[:, 0:512], pq, bqk_sb[:, mf:mf + 1]
                )
                return
            for u in range(4):
                nc.tensor.matmul(
                    pq,
                    wqk_sb[mf][:, 2 * u:2 * u + 2, :],
                    x8[u],
                    start=(u == 0),
                    stop=(u == 3),
                    perf_mode=DR,
                )
            # undo the x32 fp8 weight pre-scale (Q also carries /8): the
            # drain computes dst = pq * s + bias in one DVE op
            s = 1.0 / 256.0 if mf < 4 else 1.0 / 32.0
            nc.vector.tensor_scalar(
                dst[:, ts * 512:(ts + 1) * 512], pq,
                s, bqk_sb[:, mf:mf + 1], MUL, ADD,
            )

        def v_group(ts, xs, tt4):
            """One V projection matmul group (8 chained, bf16)."""
            xb = xs[1]
            tt = ts * 4 + tt4
            pv = psf.tile([128, 512], f32, tag="f", name="pv")
            for cc in range(NCC):
                nc.tensor.matmul(
                    pv,
                    xb[cc][:, tt4 * 128:(tt4 + 1) * 128],
                    wv_sb[cc],
                    start=(cc == 0),
                    stop=(cc == NCC - 1),
                )
            nc.vector.tensor_copy(
                out=V[tt][:, :, 0:64],
                in_=pv.rearrange("p (h d) -> p h d", h=8),
            )

        def qkv_groups(ts, xs, skip_first6=False):
            """Matmul groups of the QKV projection for t-span ts, ordered so
            pair p's QT/KT and the span's V come up in consumption order."""
            order = [("qk", 0), ("qk", 4), ("v", 0), ("v", 1), ("v", 2),
                     ("v", 3), ("qk", 1), ("qk", 5), ("qk", 2), ("qk", 6),
                     ("qk", 3), ("qk", 7)]
            if skip_first6:
                order = order[6:]
            for kind, i in order:
                if kind == "qk":
                    yield lambda i=i: qk_group(ts, xs, i)
                else:
                    yield lambda i=i: v_group(ts, xs, i)

        def proj_group(ts, mo):
            """One output-projection matmul group (4 chained, bf16)."""
            po = psf.tile([128, 512], f32, tag="f", name="po")
            for cc in range(4):
                nc.tensor.matmul(
                    po,
                    wp_sb[cc][:, mo * 128:(mo + 1) * 128],
                    Y[cc][:, ts * 512:(ts + 1) * 512],
                    start=(cc == 0),
                    stop=(cc == 3),
                )
            ot = opool.tile([128, 512], bf16, tag="ot", name="ot")
            nc.vector.tensor_copy(ot, po)
            nc.sync.dma_start(
                out=outT[mo * 128:(mo + 1) * 128, ts * 512:(ts + 1) * 512],
                in_=ot,
            )

        def proj_groups(ts):
            for mo in range(8):
                yield lambda mo=mo: proj_group(ts, mo)

        class Fillers:
            """One list per segment; pair prologues draw 2 at each boundary,
            the rest are spread proportionally over kt iterations."""

            def __init__(self, groups, n_iter, reserve=0):
                self.groups = list(groups)
                self.n_iter = max(1, n_iter)
                self.emitted = 0
                self.bdry = 0
                self.it = 0
                self.spread = max(0, len(self.groups) - 8 - reserve)

            def boundary(self, k=2):
                for _ in range(k):
                    if self.emitted < len(self.groups):
                        self.groups[self.emitted]()
                        self.emitted += 1
                        self.bdry += 1

            def step(self):
                self.it += 1
                want = self.bdry + (self.it * self.spread) // self.n_iter
                while self.emitted < min(want, len(self.groups)):
                    self.groups[self.emitted]()
                    self.emitted += 1

            def drain(self):
                while self.emitted < len(self.groups):
                    self.groups[self.emitted]()
                    self.emitted += 1

        def pair_attention(qb, p, fil):
            """Attention for q-block qb, head pair p, single-stream with a
            2-deep S pipeline: per kt the PE does PV(kt) then S(kt+2),
            ACT does exp(kt+1)."""
            q0 = 512 * qb
            n_kt = 4 * qb + 4
            y_ps = psy.tile([65, 1024], f32, tag="y", name="y_ps")

            def s_stage(kt):
                s_t = pss.tile([128, 2, 512], f32, tag="s", name="s_t")
                c_lo = max(0, 128 * kt - q0)
                d0 = 128 * kt - q0
                diag = d0 >= 0
                nc.tensor.matmul(
                    s_t[:, 0, c_lo:512],
                    KT[p][0:64, 128 * kt:128 * kt + 128],
                    QT[p][0:64, q0 + c_lo:q0 + 512],
                    start=True, stop=not diag,
                )
                nc.tensor.matmul(
                    s_t[:, 1, c_lo:512],
                    KT[p][64:128, 128 * kt:128 * kt + 128],
                    QT[p][64:128, q0 + c_lo:q0 + 512],
                    start=True, stop=not diag,
                )
                if diag:
                    nc.tensor.matmul(
                        s_t[:, 0, d0:d0 + 128], idn_sb, msk_sb,
                        start=False, stop=True,
                    )
                    nc.tensor.matmul(
                        s_t[:, 1, d0:d0 + 128], idn_sb, msk_sb,
                        start=False, stop=True,
                    )
                return s_t

            def exp_stage(kt, s_t):
                c_lo = max(0, 128 * kt - q0)
                p_t = ppool.tile([128, 2, 512], bf16, tag="p", name="p_t")
                if mode != "noexp":
                    nc.scalar.activation(
                        out=p_t[:, :, c_lo:], in_=s_t[:, :, c_lo:], func=Exp
                    )
                else:
                    nc.vector.memset(p_t[:, :, c_lo:], 0.5)
                return p_t

            def pv_stage(kt, p_t):
                c_lo = max(0, 128 * kt - q0)
                nc.tensor.matmul(
                    y_ps[:, c_lo:512], V[kt][:, 2 * p, :], p_t[:, 0, c_lo:],
                    start=(kt == 0), stop=(kt == n_kt - 1),
                )
                nc.tensor.matmul(
                    y_ps[:, 512 + c_lo:1024], V[kt][:, 2 * p + 1, :],
                    p_t[:, 1, c_lo:],
                    start=(kt == 0), stop=(kt == n_kt - 1),
                )

            # prologue: boundary fillers cover prev pair's y-psum reuse and
            # this pair's S0 -> exp0 -> PV0 latency
            fil.boundary(2)
            s_tiles = [None] * n_kt
            p_tiles = [None] * n_kt
            s_tiles[0] = s_stage(0)
            p_tiles[0] = exp_stage(0, s_tiles[0])
            if n_kt > 1:
                s_tiles[1] = s_stage(1)
            for kt in range(n_kt):
                if kt + 1 < n_kt:
                    p_tiles[kt + 1] = exp_stage(kt + 1, s_tiles[kt + 1])
                pv_stage(kt, p_tiles[kt])
                p_tiles[kt] = None
                if kt + 2 < n_kt:
                    s_tiles[kt + 2] = s_stage(kt + 2)
                    s_tiles[kt] = None
                fil.step()

            # y_ps -> SBUF copy frees the y psum quickly; then normalize:
            # y = y_un * bcast(1/den). The very last pair skips the copy
            # (nothing reuses the y psum, and the tail waits on this chain).
            if qb == NTS - 1 and p == 3:
                ycp = y_ps
            else:
                ycp = npool.tile([65, 1024], f32, tag="ycp", name="ycp")
                nc.vector.tensor_copy(ycp, y_ps)
            rb = npool.tile([64, 1024], f32, tag="rb", name="rb")
            nc.vector.reciprocal(rb[0:1, :], ycp[64:65, :])
            nc.gpsimd.partition_broadcast(rb, rb[0:1, :])
            nc.vector.tensor_mul(
                Y[p][0:64, q0:q0 + 512], ycp[0:64, 0:512], rb[:, 0:512]
            )
            nc.vector.tensor_mul(
                Y[p][64:128, q0:q0 + 512], ycp[0:64, 512:1024], rb[:, 512:1024]
            )

        # ---- prologue + pipelined segments ----
        def body(xs0):
            xs = xs0 if xs0 is not None else load_xspan(0)
            # span-0 groups needed by seg0 pair0: qk0, qk4, v0..v3
            pro = list(qkv_groups(0, xs))
            for g in pro[:6]:
                g()
            for qb in range(NTS):
                groups = []
                if qb == 0:
                    groups.extend(pro[6:])
                if qb + 1 < NTS:
                    xs = load_xspan(qb + 1)
                    if qb == 0:
                        load_wp()
                    groups.extend(qkv_groups(qb + 1, xs))
                else:
                    for ts in range(3):
                        groups.extend(proj_groups(ts))
                fil = Fillers(groups, 4 * (4 * qb + 4),
                              reserve=4 if qb == NTS - 1 else 0)
                for pr in range(4):
                    pair_attention(qb, pr, fil)
                fil.drain()
            for g in proj_groups(3):
                g()

        if reps == 1:
            body(xs)
        else:
            ET = mybir.EngineType
            with tc.For_i(0, reps, 1,
                          hint_engines=(ET.PE, ET.DVE, ET.Activation, ET.SP, ET.Pool)):
                body(None)

    nc.finalize()
    return nc


def _prep_core_inputs(x, w_attn, b_attn, w_proj, core):
    import ml_dtypes

    fp8 = ml_dtypes.float8_e4m3
    bf16 = ml_dtypes.bfloat16
    b, hg = core // 2, core % 2
    s = hg * 512
    xT = np.ascontiguousarray(x[b].T)
    # x32 pre-scale moves the small Q/K weights into fp8e4m3's normal range
    wq = w_attn[s:s + 512] * 32.0
    wk = w_attn[1024 + s:1024 + s + 512] * 32.0
    wqk = np.concatenate([wq, wk], axis=0).T  # [c, f]
    wqkg = np.ascontiguousarray(
        wqk.reshape(8, 128, 8, 128).transpose(2, 1, 0, 3))
    wqkb = np.concatenate(
        [w_attn[s:s + 512] * 0.125, w_attn[1024 + s:1024 + s + 512]],
        axis=0).T
    wqkbg = np.ascontiguousarray(
        wqkb.reshape(8, 128, 8, 128).transpose(2, 1, 0, 3))
    wv = np.ascontiguousarray(w_attn[2048 + s:2048 + s + 512].T)
    wp = np.ascontiguousarray(w_proj[:, s:s + 512].T)
    bq = b_attn[s:s + 512] * 0.125
    bk = b_attn[1024 + s:1024 + s + 512]
    bqk = np.ascontiguousarray(np.concatenate([bq, bk]).reshape(8, 128).T)
    r = np.arange(128, dtype=np.int32)
    msk = np.where(r[:, None] <= r[None, :], 0.0, -1e30)
    idn = np.eye(128)
    return {
        "xTf8": xT.astype(fp8),
        "xTbf": xT.astype(bf16),
        "wqkg": wqkg.astype(fp8),
        "wqkbg": wqkbg.astype(bf16),
        "wv": wv.astype(bf16),
        "wp": wp.astype(bf16),
        "bqk": bqk.astype(np.float32),
        "msk": msk.astype(bf16),
        "idn": idn.astype(bf16),
    }


def kernel(x, w_attn, b_attn, w_proj, b_proj):
    from concourse.bass_utils import run_bass_kernel_spmd

    x = np.asarray(x, dtype=np.float32)
    w_attn = np.asarray(w_attn, dtype=np.float32)
    b_attn = np.asarray(b_attn, dtype=np.float32)
    w_proj = np.asarray(w_proj, dtype=np.float32)
    b_proj = np.asarray(b_proj, dtype=np.float32)

    if "nc" not in _CACHE:
        _CACHE["nc"] = _build_nc()
    nc = _CACHE["nc"]

    in_maps = [
        _prep_core_inputs(x, w_attn, b_attn, w_proj, core) for core in range(8)
    ]
    res = run_bass_kernel_spmd(nc, in_maps, list(range(8)))

    # total bias: b_proj plus the (token-constant) V-bias contribution
    bias = b_proj + w_proj @ b_attn[2048:]
    out = np.empty((B, T, C), dtype=np.float32)
    for b in range(B):
        acc = (res.results[2 * b]["outT"].astype(np.float32)
               + res.results[2 * b + 1]["outT"].astype(np.float32))
        out[b] = acc.T + bias
    return out


# revision 7
# speedup vs baseline: 1.2923x; 1.0852x over previous
"""Causal self-attention (B=4, T=2048, C=1024, H=16) on 8 TRN2 NeuronCores.

Sharding: core = 2*b + hg handles batch b and heads hg*8..hg*8+8 (hybrid
batch x tensor parallel). Each core computes QKV for its heads, causal
attention, and a partial output projection over its 512 y-columns.
Host sums the two partials per batch, divides by 32 (fp8 weight
pre-scale), transposes, and adds b_proj + w_proj @ b_v.

v3: the Q/K projections for token spans 1..3 run in fp8e4m3 with the
DoubleRow perf mode (0.5 cycles/row; both operands carry 2 contraction
rows per partition as a [128, 2, N] AP) — those QKV groups drop 4x in
PE time. fp8 is ONLY safe on the score path AND only for long rows:
softmax renormalization damps score errors by ~sqrt(row length), so
~5% per-element fp8 noise in Q/K becomes <0.4% in y for tokens >=512,
but stays ~2.5% for early tokens (measured 2.9e-2 rel err with span0
in fp8 too, vs the 2e-2 tolerance; 4.9e-2 with everything fp8). So
span 0 Q/K runs in bf16, and the V path and output projection stay
bf16 (their errors reach the output undamped). x is staged twice (fp8
for spans 1-3 Q/K, bf16 for V and span0). fp8 Q/K weights are
pre-scaled x32 into the fp8 normal range; the fp8 drains rescale with
a fused tensor_scalar (mult + per-partition bias add).

v2 design notes (vs the v1 kernel):
  * bf16/fp8 device inputs (DMA is a single serialized ~360GB/s stream,
    so fewer bytes shorten the startup critical path; bf16/fp8 matmuls
    also run 1 cycle/row at ANY free size, while f32r pays 4x below
    256 free — the diagonal S blocks).
  * single-stream attention per head pair with a 2-deep S-tile pipeline
    (S(kt+2) issued while exp(kt+1) runs and PV(kt) consumes).
  * causal mask applied on PE as an extra accumulated matmul
    (idn^T @ msk into the diagonal 128-col strip of the S psum tile).
  * dedicated PSUM pools: S 2x2 banks, y 2 banks, fillers 2x1 banks.
  * DMA strictly ordered by first use; wqk grouped by feature block so
    the first QKV groups need only 2 of the 8 wqk tiles.
  * y_ps copied PSUM->SBUF right after the last PV so the y psum frees
    in ~1.2us instead of after the whole norm chain.
  * exp table preloaded at t~0; V ones-columns prefilled once.
  * fillers: seg0 carries span0-leftovers + span1 QKV, seg1/2 carry
    span2/3 QKV, seg3 carries ALL of proj0..2 (it is ACT-heavy);
    proj3 is the tail.

Device-side layouts (t=token, c=embed, f=qkv feature, d=head dim):
  xTf8 [1024, 2048]     x[b].T                             (fp8e4)
  xTbf [1024, 2048]     x[b].T                             (bf16)
  wqkg [8, 128, 8, 128] wqkg[mf][i, cc, j] = wqk[cc*128+i, mf*128+j],
                        wqk cols: 512 Q (x32 net), 512 K (x32) (fp8e4)
  wqkbg [8, 128, 8, 128] same layout, unscaled (Q /8)       (bf16)
  wv   [1024, 512]                                          (bf16)
  wp   [512, 1024]      w_proj[:, my_cols].T                (bf16)
  bqk  [128, 8]         per f-tile bias columns (Q part /8)  (f32)
  msk  [128, 128]       0 where i<=j else -1e30              (bf16)
  idn  [128, 128]       identity                             (bf16)
Output: outT [1024, 2048] partial out^T (no bias)            (f32)

Attention per head pair p (heads 2p, 2p+1 on partitions 0:64 / 64:128
of QT[p], KT[p]); per q-block qb (512 q), per kt (128 k):
  S^T[tk, tq] = K_blk @ Q^T      (2 row-packed K=64 bf16 matmuls)
  (+ diag strip: idn^T @ msk accumulated in psum — causal mask)
  P = exp(S^T) -> bf16           (no max subtraction; scores ~ N(0,1))
  [y_un^T; den] = V_aug^T @ P    (V_aug bf16 with a ones column, M=65)
  y^T = y_un^T * partition_broadcast(1/den)     (at pair end)
Projection: out^T[o, t] = wp^T @ y^T accumulated over 4 c-chunks.
"""
from contextlib import ExitStack

import numpy as np

N_HEAD = 16
C = 1024
B = 4
T = 2048
D = 64
NCC = C // 128  # c chunks
NTT = T // 128  # t tiles
NTS = T // 512  # t spans / q blocks

_CACHE = {}


def _build_nc(reps=1, mode="full", pbufs=3):
    import concourse.mybir as mybir
    import concourse.tile as tile
    from concourse import bacc

    f32 = mybir.dt.float32
    bf16 = mybir.dt.bfloat16
    fp8 = mybir.dt.float8e4
    DR = mybir.MatmulPerfMode.DoubleRow
    MUL = mybir.AluOpType.mult
    ADD = mybir.AluOpType.add

    nc = bacc.Bacc()
    # x pre-paired by c-chunk pairs on the host: one DMA per tile
    # xg*[ts][u][k, i, j] = xT[(2u+i)*128 + k, ts*512 + j]
    xgf8 = nc.declare_dram_parameter("xgf8", [NTS, 4, 128, 2, 512], fp8, isOutput=False)
    xgbf = nc.declare_dram_parameter("xgbf", [NTS, 4, 128, 2, 512], bf16, isOutput=False)
    wqkg = nc.declare_dram_parameter("wqkg", [8, 128, 8, 128], fp8, isOutput=False)
    wqkbg = nc.declare_dram_parameter("wqkbg", [8, 128, 8, 128], bf16, isOutput=False)
    wv = nc.declare_dram_parameter("wv", [C, 512], bf16, isOutput=False)
    wp = nc.declare_dram_parameter("wp", [512, C], bf16, isOutput=False)
    bqk = nc.declare_dram_parameter("bqk", [128, 8], f32, isOutput=False)
    msk = nc.declare_dram_parameter("msk", [128, 128], bf16, isOutput=False)
    idn = nc.declare_dram_parameter("idn", [128, 128], bf16, isOutput=False)
    outT = nc.declare_dram_parameter("outT", [C, T], bf16, isOutput=True)

    Exp = mybir.ActivationFunctionType.Exp

    with tile.TileContext(nc) as tc, ExitStack() as ctx:
        persist = ctx.enter_context(tc.tile_pool(name="persist", bufs=1))
        wpool = ctx.enter_context(tc.tile_pool(name="wpool", bufs=1))
        xpool = ctx.enter_context(tc.tile_pool(name="xpool", bufs=2))
        ppool = ctx.enter_context(tc.tile_pool(name="ppool", bufs=pbufs))
        npool = ctx.enter_context(tc.tile_pool(name="npool", bufs=1))
        opool = ctx.enter_context(tc.tile_pool(name="opool", bufs=2))
        pss = ctx.enter_context(tc.tile_pool(name="pss", bufs=2, space="PSUM"))
        psy = ctx.enter_context(tc.tile_pool(name="psy", bufs=1, space="PSUM"))
        psf = ctx.enter_context(tc.tile_pool(name="psf", bufs=2, space="PSUM"))

        bqk_sb = persist.tile([128, 8], f32)
        msk_sb = persist.tile([128, 128], bf16)
        idn_sb = persist.tile([128, 128], bf16)
        warm = persist.tile([128, 1], f32)
        warm_o = persist.tile([128, 1], bf16)

        QT = [persist.tile([128, T], bf16, tag=f"qt{p}", name=f"qt{p}") for p in range(4)]
        KT = [persist.tile([128, T], bf16, tag=f"kt{p}", name=f"kt{p}") for p in range(4)]
        V = [persist.tile([128, 8, 65], bf16, tag=f"v{tt}", name=f"v{tt}") for tt in range(NTT)]
        Y = [persist.tile([128, T], bf16, tag=f"y{p}", name=f"y{p}") for p in range(4)]

        # preload the Exp activation table + V ones-columns while DMAs run
        nc.vector.memset(warm, 0.0)
        nc.scalar.activation(out=warm_o, in_=warm, func=Exp)
        for tt in range(NTT):
            nc.vector.memset(V[tt][:, :, 64:65], 1.0)

        # ---- DMA priming, strictly in first-use order ----
        wqk_sb = [None] * 8
        wqkb_sb = [None] * 8

        def load_wqk(mf):
            t_ = wpool.tile([128, 8, 128], fp8, tag=f"wqk{mf}", name=f"wqk{mf}")
            nc.sync.dma_start(out=t_, in_=wqkg[mf, :, :, :])
            wqk_sb[mf] = t_

        def load_wqkb(mf):
            t_ = wpool.tile([128, 8, 128], bf16, tag=f"wqkb{mf}", name=f"wqkb{mf}")
            nc.sync.dma_start(out=t_, in_=wqkbg[mf, :, :, :])
            wqkb_sb[mf] = t_

        load_wqkb(0)

        def load_xspan(ts):
            x8 = []
            for u in range(4 if ts > 0 else 0):
                t_ = xpool.tile([128, 2, 512], fp8, tag=f"x8{u}", name=f"x8{u}")
                nc.sync.dma_start(out=t_, in_=xgf8[ts, u, :, :, :])
                x8.append(t_)
            xb2 = []
            for u in range(4):
                t_ = xpool.tile([128, 2, 512], bf16, tag=f"xb{u}", name=f"xb{u}")
                nc.sync.dma_start(out=t_, in_=xgbf[ts, u, :, :, :])
                xb2.append(t_)
            xb = [xb2[cc // 2][:, cc % 2, :] for cc in range(NCC)]
            return x8, xb

        xs = load_xspan(0)
        load_wqkb(4)
        nc.sync.dma_start(out=bqk_sb, in_=bqk[:, :])
        nc.sync.dma_start(out=msk_sb, in_=msk[:, :])
        nc.sync.dma_start(out=idn_sb, in_=idn[:, :])
        wv_sb = []
        for cc in range(NCC):
            t_ = wpool.tile([128, 512], bf16, tag=f"wv{cc}", name=f"wv{cc}")
            nc.sync.dma_start(out=t_, in_=wv[cc * 128:(cc + 1) * 128, :])
            wv_sb.append(t_)
        for mf in (1, 5, 2, 6, 3, 7):
            load_wqkb(mf)
        for mf in range(8):
            load_wqk(mf)

        wp_sb = []

        def load_wp():
            for cc in range(4):
                t_ = wpool.tile([128, 1024], bf16, tag=f"wp{cc}", name=f"wp{cc}")
                nc.sync.dma_start(out=t_, in_=wp[cc * 128:(cc + 1) * 128, :])
                wp_sb.append(t_)

        def qk_group(ts, xs, mf):
            """One QT/KT projection group. Span 0 runs bf16 (8 chained
            matmuls — short attention rows can't afford fp8 noise);
            spans 1-3 run fp8 DoubleRow (4 chained)."""
            x8, xb = xs
            pq = psf.tile([128, 512], f32, tag="f", name="pq")
            dst = QT[mf] if mf < 4 else KT[mf - 4]
            if ts == 0:
                for cc in range(NCC):
                    nc.tensor.matmul(
                        pq,
                        wqkb_sb[mf][:, cc, :],
                        xb[cc],
                        start=(cc == 0),
                        stop=(cc == NCC - 1),
                    )
                nc.vector.tensor_scalar_add(
                    dst[:, 0:512], pq, bqk_sb[:, mf:mf + 1]
                )
                return
            for u in range(4):
                nc.tensor.matmul(
                    pq,
                    wqk_sb[mf][:, 2 * u:2 * u + 2, :],
                    x8[u],
                    start=(u == 0),
                    stop=(u == 3),
                    perf_mode=DR,
                )
            # undo the x32 fp8 weight pre-scale (Q also carries /8): the
            # drain computes dst = pq * s + bias in one DVE op
            s = 1.0 / 256.0 if mf < 4 else 1.0 / 32.0
            nc.vector.tensor_scalar(
                dst[:, ts * 512:(ts + 1) * 512], pq,
                s, bqk_sb[:, mf:mf + 1], MUL, ADD,
            )

        def v_group(ts, xs, tt4):
            """One V projection matmul group (8 chained, bf16)."""
            xb = xs[1]
            tt = ts * 4 + tt4
            pv = psf.tile([128, 512], f32, tag="f", name="pv")
            for cc in range(NCC):
                nc.tensor.matmul(
                    pv,
                    xb[cc][:, tt4 * 128:(tt4 + 1) * 128],
                    wv_sb[cc],
                    start=(cc == 0),
                    stop=(cc == NCC - 1),
                )
            nc.vector.tensor_copy(
                out=V[tt][:, :, 0:64],
                in_=pv.rearrange("p (h d) -> p h d", h=8),
            )

        def qkv_groups(ts, xs, skip_first6=False):
            """Matmul groups of the QKV projection for t-span ts, ordered so
            pair p's QT/KT and the span's V come up in consumption order."""
            order = [("qk", 0), ("qk", 4), ("v", 0), ("v", 1), ("v", 2),
                     ("v", 3), ("qk", 1), ("qk", 5), ("qk", 2), ("qk", 6),
                     ("qk", 3), ("qk", 7)]
            if skip_first6:
                order = order[6:]
            for kind, i in order:
                if kind == "qk":
                    yield lambda i=i: qk_group(ts, xs, i)
                else:
                    yield lambda i=i: v_group(ts, xs, i)

        def proj_group(ts, mo):
            """One output-projection matmul group (4 chained, bf16)."""
            po = psf.tile([128, 512], f32, tag="f", name="po")
            for cc in range(4):
                nc.tensor.matmul(
                    po,
                    wp_sb[cc][:, mo * 128:(mo + 1) * 128],
                    Y[cc][:, ts * 512:(ts + 1) * 512],
                    start=(cc == 0),
                    stop=(cc == 3),
                )
            ot = opool.tile([128, 512], bf16, tag="ot", name="ot")
            nc.vector.tensor_copy(ot, po)
            nc.sync.dma_start(
                out=outT[mo * 128:(mo + 1) * 128, ts * 512:(ts + 1) * 512],
                in_=ot,
            )

        def proj_groups(ts):
            for mo in range(8):
                yield lambda mo=mo: proj_group(ts, mo)

        class Fillers:
            """One list per segment; pair prologues draw 2 at each boundary,
            the rest are spread proportionally over kt iterations."""

            def __init__(self, groups, n_iter, reserve=0):
                self.groups = list(groups)
                self.n_iter = max(1, n_iter)
                self.emitted = 0
                self.bdry = 0
                self.it = 0
                self.spread = max(0, len(self.groups) - 8 - reserve)

            def boundary(self, k=2):
                for _ in range(k):
                    if self.emitted < len(self.groups):
                        self.groups[self.emitted]()
                        self.emitted += 1
                        self.bdry += 1

            def step(self):
                self.it += 1
                want = self.bdry + (self.it * self.spread) // self.n_iter
                while self.emitted < min(want, len(self.groups)):
                    self.groups[self.emitted]()
                    self.emitted += 1

            def drain(self):
                while self.emitted < len(self.groups):
                    self.groups[self.emitted]()
                    self.emitted += 1

        def pair_attention(qb, p, fil):
            """Attention for q-block qb, head pair p, single-stream with a
            2-deep S pipeline: per kt the PE does PV(kt) then S(kt+2),
            ACT does exp(kt+1)."""
            q0 = 512 * qb
            n_kt = 4 * qb + 4
            y_ps = psy.tile([65, 1024], f32, tag="y", name="y_ps")

            def s_stage(kt):
                s_t = pss.tile([128, 2, 512], f32, tag="s", name="s_t")
                c_lo = max(0, 128 * kt - q0)
                d0 = 128 * kt - q0
                diag = d0 >= 0
                nc.tensor.matmul(
                    s_t[:, 0, c_lo:512],
                    KT[p][0:64, 128 * kt:128 * kt + 128],
                    QT[p][0:64, q0 + c_lo:q0 + 512],
                    start=True, stop=not diag,
                )
                nc.tensor.matmul(
                    s_t[:, 1, c_lo:512],
                    KT[p][64:128, 128 * kt:128 * kt + 128],
                    QT[p][64:128, q0 + c_lo:q0 + 512],
                    start=True, stop=not diag,
                )
                if diag:
                    nc.tensor.matmul(
                        s_t[:, 0, d0:d0 + 128], idn_sb, msk_sb,
                        start=False, stop=True,
                    )
                    nc.tensor.matmul(
                        s_t[:, 1, d0:d0 + 128], idn_sb, msk_sb,
                        start=False, stop=True,
                    )
                return s_t

            def exp_stage(kt, s_t):
                c_lo = max(0, 128 * kt - q0)
                p_t = ppool.tile([128, 2, 512], bf16, tag="p", name="p_t")
                if mode != "noexp":
                    nc.scalar.activation(
                        out=p_t[:, :, c_lo:], in_=s_t[:, :, c_lo:], func=Exp
                    )
                else:
                    nc.vector.memset(p_t[:, :, c_lo:], 0.5)
                return p_t

            def pv_stage(kt, p_t):
                c_lo = max(0, 128 * kt - q0)
                nc.tensor.matmul(
                    y_ps[:, c_lo:512], V[kt][:, 2 * p, :], p_t[:, 0, c_lo:],
                    start=(kt == 0), stop=(kt == n_kt - 1),
                )
                nc.tensor.matmul(
                    y_ps[:, 512 + c_lo:1024], V[kt][:, 2 * p + 1, :],
                    p_t[:, 1, c_lo:],
                    start=(kt == 0), stop=(kt == n_kt - 1),
                )

            # prologue: boundary fillers cover prev pair's y-psum reuse and
            # this pair's S0 -> exp0 -> PV0 latency
            fil.boundary(2)
            s_tiles = [None] * n_kt
            p_tiles = [None] * n_kt
            s_tiles[0] = s_stage(0)
            p_tiles[0] = exp_stage(0, s_tiles[0])
            if n_kt > 1:
                s_tiles[1] = s_stage(1)
            for kt in range(n_kt):
                if kt + 1 < n_kt:
                    p_tiles[kt + 1] = exp_stage(kt + 1, s_tiles[kt + 1])
                pv_stage(kt, p_tiles[kt])
                p_tiles[kt] = None
                if kt + 2 < n_kt:
                    s_tiles[kt + 2] = s_stage(kt + 2)
                    s_tiles[kt] = None
                fil.step()

            # y_ps -> SBUF copy frees the y psum quickly; then normalize:
            # y = y_un * bcast(1/den). The very last pair skips the copy
            # (nothing reuses the y psum, and the tail waits on this chain).
            if qb == NTS - 1 and p == 3:
                ycp = y_ps
            else:
                ycp = npool.tile([65, 1024], f32, tag="ycp", name="ycp")
                nc.vector.tensor_copy(ycp, y_ps)
            rb = npool.tile([64, 1024], f32, tag="rb", name="rb")
            nc.vector.reciprocal(rb[0:1, :], ycp[64:65, :])
            nc.gpsimd.partition_broadcast(rb, rb[0:1, :])
            nc.vector.tensor_mul(
                Y[p][0:64, q0:q0 + 512], ycp[0:64, 0:512], rb[:, 0:512]
            )
            nc.vector.tensor_mul(
                Y[p][64:128, q0:q0 + 512], ycp[0:64, 512:1024], rb[:, 512:1024]
            )

        # ---- prologue + pipelined segments ----
        def body(xs0):
            xs = xs0 if xs0 is not None else load_xspan(0)
            # span-0 groups needed by seg0 pair0: qk0, qk4, v0..v3
            pro = list(qkv_groups(0, xs))
            for g in pro[:6]:
                g()
            for qb in range(NTS):
                groups = []
                if qb == 0:
                    groups.extend(pro[6:])
                if qb + 1 < NTS:
                    xs = load_xspan(qb + 1)
                    if qb == 0:
                        load_wp()
                    groups.extend(qkv_groups(qb + 1, xs))
                else:
                    for ts in range(3):
                        groups.extend(proj_groups(ts))
                fil = Fillers(groups, 4 * (4 * qb + 4),
                              reserve=4 if qb == NTS - 1 else 0)
                for pr in range(4):
                    pair_attention(qb, pr, fil)
                fil.drain()
            for g in proj_groups(3):
                g()

        if reps == 1:
            body(xs)
        else:
            ET = mybir.EngineType
            with tc.For_i(0, reps, 1,
                          hint_engines=(ET.PE, ET.DVE, ET.Activation, ET.SP, ET.Pool)):
                body(None)

    nc.finalize()
    return nc


def _prep_core_inputs(x, w_attn, b_attn, w_proj, core):
    import ml_dtypes

    fp8 = ml_dtypes.float8_e4m3
    bf16 = ml_dtypes.bfloat16
    b, hg = core // 2, core % 2
    s = hg * 512
    xT = np.ascontiguousarray(x[b].T)
    # [ts, u, k, i, j] = xT[(2u+i)*128 + k, ts*512 + j]
    xg = np.ascontiguousarray(
        xT.reshape(4, 2, 128, 4, 512).transpose(3, 0, 2, 1, 4))
    # x32 pre-scale moves the small Q/K weights into fp8e4m3's normal range
    wq = w_attn[s:s + 512] * 32.0
    wk = w_attn[1024 + s:1024 + s + 512] * 32.0
    wqk = np.concatenate([wq, wk], axis=0).T  # [c, f]
    wqkg = np.ascontiguousarray(
        wqk.reshape(8, 128, 8, 128).transpose(2, 1, 0, 3))
    wqkb = np.concatenate(
        [w_attn[s:s + 512] * 0.125, w_attn[1024 + s:1024 + s + 512]],
        axis=0).T
    wqkbg = np.ascontiguousarray(
        wqkb.reshape(8, 128, 8, 128).transpose(2, 1, 0, 3))
    wv = np.ascontiguousarray(w_attn[2048 + s:2048 + s + 512].T)
    wp = np.ascontiguousarray(w_proj[:, s:s + 512].T)
    bq = b_attn[s:s + 512] * 0.125
    bk = b_attn[1024 + s:1024 + s + 512]
    bqk = np.ascontiguousarray(np.concatenate([bq, bk]).reshape(8, 128).T)
    r = np.arange(128, dtype=np.int32)
    msk = np.where(r[:, None] <= r[None, :], 0.0, -1e30)
    idn = np.eye(128)
    return {
        "xgf8": xg.astype(fp8),
        "xgbf": xg.astype(bf16),
        "wqkg": wqkg.astype(fp8),
        "wqkbg": wqkbg.astype(bf16),
        "wv": wv.astype(bf16),
        "wp": wp.astype(bf16),
        "bqk": bqk.astype(np.float32),
        "msk": msk.astype(bf16),
        "idn": idn.astype(bf16),
    }


def kernel(x, w_attn, b_attn, w_proj, b_proj):
    from concourse.bass_utils import run_bass_kernel_spmd

    x = np.asarray(x, dtype=np.float32)
    w_attn = np.asarray(w_attn, dtype=np.float32)
    b_attn = np.asarray(b_attn, dtype=np.float32)
    w_proj = np.asarray(w_proj, dtype=np.float32)
    b_proj = np.asarray(b_proj, dtype=np.float32)

    if "nc" not in _CACHE:
        _CACHE["nc"] = _build_nc()
    nc = _CACHE["nc"]

    in_maps = [
        _prep_core_inputs(x, w_attn, b_attn, w_proj, core) for core in range(8)
    ]
    res = run_bass_kernel_spmd(nc, in_maps, list(range(8)))

    # total bias: b_proj plus the (token-constant) V-bias contribution
    bias = b_proj + w_proj @ b_attn[2048:]
    out = np.empty((B, T, C), dtype=np.float32)
    for b in range(B):
        acc = (res.results[2 * b]["outT"].astype(np.float32)
               + res.results[2 * b + 1]["outT"].astype(np.float32))
        out[b] = acc.T + bias
    return out
